# revision 1
# baseline (speedup 1.0000x reference)
"""GAT layer (nn_GAT_layer_67619965108552) as a Trainium2 Bass/Tile SPMD kernel.

Structure exploited (same math as the verified baseline):
  With n=8192, the buggy-but-faithful pair indexing collapses:
    rows i < 4096:  scores[i, j] = u[2i + (j >= 4096)],  u = x @ (W@a1 + W@a2)
    rows i >= 4096: scores[i, j] = tt[j mod 4096],       tt = s1[even] + s2[odd]
  After leaky_relu + adj masking + softmax, attn @ out reduces to two masked
  row-sum matmuls against adj halves:
    Y1 = A[:, :4096] @ [f*out_L | f],  Y2 = A[:, 4096:] @ [f*out_R | f]
    res = sigmoid((al1*Y1 + al2*Y2)[:, :256] / (al1*Y1 + al2*Y2)[:, 256])
  Top-half cores: f = 1, al1 = exp(lrelu(u_even)), al2 = exp(lrelu(u_odd));
  bottom-half cores: f = exp(lrelu(tt)), al1 = al2 = 1. Same instruction
  stream on all cores; only input data (g / select masks) differs.

Layout strategy (this is where the speed comes from vs the old version):
  * x arrives host-pre-transposed as bf16 xT [512, 8192] -> matmul lhsT/rhs
    tiles load straight from DRAM; zero on-chip transposes.
  * each core's adj slice arrives host-pre-transposed as int8 adjT [8192,1024]
    -> SWDGE dma casts i8->bf16 on load; tiles are directly the lhsT of the
    Y matmuls; zero on-chip transposes and 4x less HBM traffic.
  * U-pass (scores) runs before the out-pass; score vectors scatter to a DRAM
    scratch DURING the U-pass and reload once partition-major, so leaky_relu/
    exp run on [128, 96] tiles and the tensor engine never idles waiting.
  * Stage B accumulates Y in 8 PSUM banks per half directly.

Sharding: rows of adj (and of the output) across 8 cores, 1024 rows each.
x/weight/att_vec replicated; every core computes the full out = x@W.
"""
import numpy as np
from contextlib import ExitStack

import concourse.bass as bass
import concourse.tile as tile
from concourse import bacc, mybir
from concourse.bass_utils import run_bass_kernel_spmd

F32 = mybir.dt.float32
F32R = mybir.dt.float32r
BF16 = mybir.dt.bfloat16
F8 = mybir.dt.float8e4
I8 = mybir.dt.int8

N = 8192          # nodes
FIN = 512         # input features
FOUT = 256        # output features
P = 128
NB = N // P       # 64 j-chunks over all nodes
NCORES = 8
RPC = N // NCORES  # 1024 rows per core
MB = RPC // P      # 8 output row-blocks per core
HKC = 32           # j-chunks per half (4096/128)
GJ = 8             # j-chunks per adj DMA group
NG = NB // GJ      # 8 adj groups
UG = 16            # U-pass groups of 512 nodes


def build_program():
    nc = bacc.Bacc("TRN2", target_bir_lowering=False, debug=False,
                   num_devices=NCORES)

    xt_d = nc.dram_tensor("xt", [FIN, N], F8, kind="ExternalInput")
    wt_d = nc.dram_tensor("wt", [FOUT, FIN], F32, kind="ExternalInput")
    w8_d = nc.dram_tensor("w8", [FIN, FOUT], F8, kind="ExternalInput")
    att3_d = nc.dram_tensor("att3", [FOUT, 3], F32, kind="ExternalInput")
    adjt_d = nc.dram_tensor("adjt", [N, RPC], I8, kind="ExternalInput")
    # gcol[:, 0] = g (1.0 for top-half cores, 0.0 for bottom), gcol[:, 1] = 1-g
    g_d = nc.dram_tensor("gcol", [P, 2], F32, kind="ExternalInput")
    # selg[p, B', B] = g * (B == 8c + B') : per-core row-block select
    selg_d = nc.dram_tensor("selg", [P, MB, HKC], F32, kind="ExternalInput")
    # basis vectors for psum row extraction: col 0 -> row 1, col 1 -> row 2
    eb_d = nc.dram_tensor("ebasis", [3, 2], F32R, kind="ExternalInput")
    y_d = nc.dram_tensor("y", [RPC, FOUT], F32, kind="ExternalOutput")

    with tile.TileContext(nc) as tc, ExitStack() as ctx:
        constp = ctx.enter_context(tc.tile_pool(name="const", bufs=1))
        dramp = ctx.enter_context(tc.tile_pool(name="dram", bufs=1, space="DRAM"))
        # adj tiles stream through 3 slots; DMAs issued up-front so they run
        # under stage A (gpsimd/SWDGE queue carries nothing else).
        adjp = ctx.enter_context(tc.tile_pool(name="adjp", bufs=3))

        # ---- constants (scalar HWDGE ring, so the sync ring starts
        # streaming xT immediately) ----
        wt = constp.tile([P, 2, FIN], F32)         # W^T, f-chunk major
        nc.scalar.dma_start(wt[:], wt_d.ap().rearrange("(c p) k -> p c k", p=P))
        att3 = constp.tile([P, 2, 3], F32)         # [a1+a2 | a1 | a2] cols
        nc.scalar.dma_start(att3[:], att3_d.ap().rearrange("(c p) v -> p c v", p=P))
        wq = constp.tile([P, 4, FOUT], F8)         # W fp8, k-chunk major
        nc.scalar.dma_start(wq[:], w8_d.ap().rearrange("(c p) f -> p c f", p=P))
        gcol = constp.tile([P, 2], F32)
        nc.scalar.dma_start(gcol[:], g_d.ap())
        selg = constp.tile([P, MB, HKC], F32)
        nc.scalar.dma_start(selg[:], selg_d.ap())
        ebasis = constp.tile([3, 2], F32R)
        nc.scalar.dma_start(ebasis[:], eb_d.ap())

        # scratch for dummy ACT-table warms (placed just ahead of each
        # table's first real use; a startup preload only delays the init
        # barrier, and a just-in-time load lands on a critical path)
        dumm = constp.tile([1, 2], F32)
        nc.gpsimd.memset(dumm[:], 0.0)

        # persistent mid-size tensors
        outb = [constp.tile([P, HKC, FOUT + 1], BF16, name=f"outb{h}")
                for h in range(2)]
        zsb = constp.tile([P, MB, FOUT + 1], F32)
        rawpm = constp.tile([P, 3 * HKC], F32)   # [ae_raw | be_raw | tt_raw]
        expv = constp.tile([P, 3 * HKC], F32)    # exp(lrelu(rawpm))
        fpm = constp.tile([P, HKC], F32)
        al1 = constp.tile([P, MB], F32)
        al2 = constp.tile([P, MB], F32)
        wam = constp.tile([P, 4, 3], F8)

        # DRAM scratch for the free-major -> partition-major shuffle of the
        # score vectors: rows = [ae | be | tt], each 4096 long, index-major.
        vecd = dramp.tile([3, N // 2], F32)

        def adj_load(g, marker=None):
            t = adjp.tile([P, GJ, RPC], BF16, tag="adjg", name=f"adjg{g}")
            if marker is not None:
                # Real dependency gate: the dataflow scheduler hoists
                # ready DMAs arbitrarily early, and an early adj transfer
                # steals HBM bandwidth from the xT stream that paces the
                # U-pass. Writing one corner element from a mid-U-pass
                # tile forces this DMA (WAW on the corner) to wait.
                nc.scalar.copy(t[0:1, 0, 0:1], marker)
            nc.gpsimd.dma_start(
                t[:],
                adjt_d.ap()[g * GJ * P:(g + 1) * GJ * P, :].rearrange(
                    "(t p) i -> p t i", p=P))
            return t

        # adj loads are all deferred until after the stage-A gather DMAs:
        # they'd compete with the xT stream for HBM (the U-pass is paced by
        # it) and anything slot-waiting on stage-B consumption must not sit
        # ahead of the gathers in the gpsimd FIFO.
        adjg = []

        # ---- stage A (scoped pools) ----
        with tc.tile_pool(name="xtp", bufs=1) as xtp, \
             tc.tile_pool(name="sa", bufs=3) as sa:

            # resident xT [k-part, c, n] as 8 slice tiles of 1 MB each, so
            # the U-pass only waits on the slices it has reached
            NS = N // 8
            xts = []
            for s in range(8):
                sl = slice(s * NS, (s + 1) * NS)
                xs = xtp.tile([P, 4, NS], F8, name=f"xt{s}")
                nc.sync.dma_start(
                    xs[:],
                    xt_d.ap()[:, sl].rearrange("(c p) n -> p c n", p=P))
                xts.append(xs)

            def xtsl(n0, n1):
                """[P, 4, n1-n0] view of the xT slice tile covering n0:n1."""
                s = n0 // NS
                assert (n1 - 1) // NS == s
                return xts[s][:, :, n0 - s * NS:n1 - s * NS]

            # wam[:, c, :] = [wu | wa1 | wa2] = W @ att3, computed on the
            # PE (a serial DVE chain here costs ~6us of kernel startup)
            with tc.tile_pool(name="ps_w", bufs=4, space="PSUM") as ps_w:
                for kc in range(4):
                    pw = ps_w.tile([P, 3], F32, tag="pw", name="pw")
                    for fc in range(2):
                        nc.tensor.matmul(pw[:],
                                         wt[:, fc, kc * P:(kc + 1) * P],
                                         att3[:, fc, :],
                                         start=(fc == 0), stop=(fc == 1))
                    nc.vector.tensor_copy(wam[:, kc, :], pw[:])

            # ---- U pass, ext extraction software-pipelined one group
            # behind so the in-order PE queue never waits on the DVE
            # PSUM->SBUF copy (which would gap the PE and keep HAM cold).
            # Its PSUM pools are scoped so the out-pass can use all 8 banks.
            ups = ctx_u = ExitStack()
            ps_u = ctx_u.enter_context(
                tc.tile_pool(name="ps_u", bufs=2, space="PSUM"))
            ps_e = ctx_u.enter_context(
                tc.tile_pool(name="ps_e", bufs=2, space="PSUM"))

            def u_mms(g):
                pu = ps_u.tile([3, 512], F32, tag="pu", name="pu")
                for c in range(4):
                    nc.tensor.matmul(pu[:], wam[:, c, :],
                                     xtsl(g * 512, (g + 1) * 512)[:, c, :],
                                     start=(c == 0), stop=(c == 3))
                pusb = sa.tile([3, 512], F32R, tag="pusb", name="pusb")
                if g % 2 == 0:
                    nc.vector.tensor_copy(pusb[:], pu[:])
                else:
                    nc.scalar.copy(pusb[:], pu[:])
                return pusb

            def u_ext(g, pusb):
                # tt = s1[even] + s2[odd]: two basis extractions into one PSUM
                ext = ps_e.tile([1, 256], F32, tag="ext", name="ext")
                nc.tensor.matmul(ext[:], ebasis[:, 0:1], pusb[:, 0::2],
                                 start=True, stop=False)
                nc.tensor.matmul(ext[:], ebasis[:, 1:2], pusb[:, 1::2],
                                 start=False, stop=True)
                # de-interleave [ae | be | tt] into a staging tile, then one
                # contiguous 3 KB store into the DRAM scratch rows (scalar
                # HWDGE ring -- sync is busy streaming xT)
                svg = sa.tile([1, 3, 256], F32, tag="svg", name="svg")
                nc.vector.tensor_copy(
                    svg[:, 0:2, :],
                    pusb[0:1, :].rearrange("r (m v) -> r v m", v=2))
                nc.vector.tensor_copy(svg[:, 2, :], ext[:])
                nc.scalar.dma_start(vecd[:, g * 256:(g + 1) * 256], svg[:])

            pend = None
            for g in range(UG):
                pusb = u_mms(g)
                if pend is not None:
                    u_ext(*pend)
                pend = (g, pusb)
            u_ext(*pend)
            ctx_u.close()
            # out-pass PSUM: all 8 banks, so copy-stream hiccups don't
            # backpressure the matmul chains
            ctx_a = ExitStack()
            ps_a = ctx_a.enter_context(
                tc.tile_pool(name="ps_a", bufs=8, space="PSUM"))

            # ---- partition-major reload (SWDGE gathers). fpm (and thus
            # the outb scaling) needs only the tt row, and its first half
            # (groups 0-7) is stored by mid-U-pass -- gather + process it
            # in halves so fpm[:, 0:16] exists ~15us before the rest ----
            HH = HKC // 2
            for hv in range(2):
                csl = slice(hv * (N // 4), (hv + 1) * (N // 4))
                rsl = slice(2 * HKC + hv * HH, 2 * HKC + (hv + 1) * HH)
                nc.gpsimd.dma_start(
                    rawpm[:, rsl],
                    vecd[2:3, csl].rearrange("r (B p) -> (r p) B", p=P))
                if hv == 0:
                    # exp-table warm ANCHORED on the first tt gather (a
                    # dep-free dummy gets hoisted to t=0 and evicted; a
                    # just-in-time load lands on the fpm critical path)
                    nc.scalar.activation(dumm[:, 0:1],
                                         rawpm[0:1, 2 * HKC:2 * HKC + 1],
                                         mybir.ActivationFunctionType.Exp)
                lrt = sa.tile([P, HH], F32, tag="lrt", name="lrt")
                nc.vector.tensor_scalar_mul(lrt[:], rawpm[:, rsl], 0.01)
                nc.vector.tensor_max(lrt[:], rawpm[:, rsl], lrt[:])
                nc.scalar.activation(expv[:, rsl], lrt[:],
                                     mybir.ActivationFunctionType.Exp)
                # f = g + (1-g)*v  (per-partition scalars from gcol)
                nc.vector.tensor_scalar(fpm[:, hv * HH:(hv + 1) * HH],
                                        expv[:, rsl],
                                        gcol[:, 1:2], gcol[:, 0:1],
                                        op0=mybir.AluOpType.mult,
                                        op1=mybir.AluOpType.add)
            # ae/be rows feed only the alphas (needed mid-stage-B)
            for v in (0, 1):
                nc.gpsimd.dma_start(
                    rawpm[:, v * HKC:(v + 1) * HKC],
                    vecd[v:v + 1, :].rearrange("r (B p) -> (r p) B", p=P))

            # adj loads: groups 0-2 gated on rawpm (i.e. after the gathers,
            # when the xT stream is done); groups 3+ are gated by their
            # pool-slot WAR on stage-B consumption anyway
            adjg.extend(adj_load(g, marker=rawpm[0:1, 0:1]) for g in range(3))
            adjg.extend(adj_load(g) for g in range(3, NG))

            # ---- out pass: out = x @ W, blocks land in outb; the f-scale
            # of block b is emitted a 16-block lag behind its copy so it
            # rides the copy-stream gaps (fpm is ready by then) instead of
            # stalling the stage-B transition ----
            for h in range(2):
                nc.vector.tensor_copy(outb[h][:, :, FOUT:FOUT + 1], fpm[:])

            def out_scale(b):
                h, kc = (0, b) if b < HKC else (1, b - HKC)
                dst = outb[h][:, kc, :FOUT]
                if b % 2 == 0:
                    nc.vector.tensor_scalar_mul(dst, dst, fpm[:, kc:kc + 1])
                else:
                    nc.scalar.activation(dst, dst,
                                         mybir.ActivationFunctionType.Copy,
                                         scale=fpm[:, kc:kc + 1])

            for b in range(NB):
                po = ps_a.tile([P, FOUT], F32, tag="po", name="po")
                xv = xtsl(b * P, (b + 1) * P)
                for c in range(4):
                    nc.tensor.matmul(po[:], xv[:, c, :], wq[:, c, :],
                                     start=(c == 0), stop=(c == 3))
                h, kc = (0, b) if b < HKC else (1, b - HKC)
                if b % 2 == 0:
                    nc.scalar.copy(outb[h][:, kc, :FOUT], po[:])
                else:
                    nc.vector.tensor_copy(outb[h][:, kc, :FOUT], po[:])
                if b >= 16:
                    out_scale(b - 16)
            for b in range(NB - 16, NB):
                out_scale(b)

            # ---- alphas (needed only by the stage-B drains; runs in the
            # out-pass -> stage-B transition when DVE/ACT go idle) ----
            lra = sa.tile([P, 2 * HKC], F32, tag="lra", name="lra")
            nc.vector.tensor_scalar_mul(lra[:], rawpm[:, 0:2 * HKC], 0.01)
            nc.vector.tensor_max(lra[:], rawpm[:, 0:2 * HKC], lra[:])
            nc.scalar.activation(expv[:, 0:2 * HKC], lra[:],
                                 mybir.ActivationFunctionType.Exp)
            # al{1,2}[:, B'] = sum_B {ae,be}[:, B]*selg[:, B', B] + 1-g
            for bp in range(MB):
                m1 = sa.tile([P, HKC], F32, tag="alm", name="alm1")
                nc.vector.tensor_mul(m1[:], expv[:, 0:HKC], selg[:, bp, :])
                nc.vector.tensor_reduce(al1[:, bp:bp + 1], m1[:],
                                        axis=mybir.AxisListType.X,
                                        op=mybir.AluOpType.add)
                m2 = sa.tile([P, HKC], F32, tag="alm", name="alm2")
                nc.vector.tensor_mul(m2[:], expv[:, HKC:2 * HKC],
                                     selg[:, bp, :])
                nc.vector.tensor_reduce(al2[:, bp:bp + 1], m2[:],
                                        axis=mybir.AxisListType.X,
                                        op=mybir.AluOpType.add)
            nc.vector.tensor_scalar_add(al1[:], al1[:], gcol[:, 1:2])
            nc.vector.tensor_scalar_add(al2[:], al2[:], gcol[:, 1:2])
            ctx_a.close()

        # ---- stage B: Y = adjT.T @ outb, 8 PSUM banks per half ----
        with tc.tile_pool(name="ps_y", bufs=8, space="PSUM") as ps_y, \
             tc.tile_pool(name="comb", bufs=3) as comb:

            for h in range(2):
                yps = [ps_y.tile([P, FOUT + 1], F32, tag="yp",
                                 name=f"yp{h}_{m}") for m in range(MB)]
                for g in range(NG // 2):
                    at = adjg[h * (NG // 2) + g]
                    for t in range(GJ):
                        jc = g * GJ + t
                        for mb in range(MB):
                            nc.tensor.matmul(
                                yps[mb][:],
                                at[:, t, mb * P:(mb + 1) * P],
                                outb[h][:, jc, :],
                                start=(jc == 0), stop=(jc == HKC - 1))
                if h == 0:
                    for mb in range(MB):
                        nc.scalar.activation(
                            zsb[:, mb, :], yps[mb][:],
                            mybir.ActivationFunctionType.Copy,
                            scale=al1[:, mb:mb + 1])
                    # sigmoid-table warm ANCHORED on the first drain so it
                    # runs mid-stage-B, not at t=0 (hoisted) nor on the
                    # critical tail (just-in-time)
                    nc.scalar.activation(dumm[:, 0:1], zsb[0:1, 0, 0:1],
                                         mybir.ActivationFunctionType.Sigmoid)
                else:
                    # per-mb pipelined epilogue: each mb's chain starts as
                    # soon as its accumulator stops, staggered across
                    # ACT (scale-copy, sigmoid) / DVE (add, recip, mul) /
                    # sync (store)
                    for mb in range(MB):
                        t2 = comb.tile([P, FOUT + 1], F32, tag="t2",
                                       name="t2")
                        if mb % 2 == 0:
                            nc.scalar.activation(
                                t2[:], yps[mb][:],
                                mybir.ActivationFunctionType.Copy,
                                scale=al2[:, mb:mb + 1])
                        else:
                            nc.vector.tensor_scalar_mul(t2[:], yps[mb][:],
                                                        al2[:, mb:mb + 1])
                        z2 = comb.tile([P, FOUT + 1], F32, tag="z2",
                                       name="z2")
                        nc.vector.tensor_add(z2[:], zsb[:, mb, :], t2[:])
                        rec = comb.tile([P, 1], F32, tag="rec", name="rec")
                        nc.vector.reciprocal(rec[:], z2[:, FOUT:FOUT + 1])
                        res = comb.tile([P, FOUT], F32, tag="res", name="res")
                        nc.vector.tensor_scalar_mul(res[:], z2[:, :FOUT],
                                                    rec[:])
                        resg = comb.tile([P, FOUT], F32, tag="resg",
                                         name="resg")
                        nc.scalar.activation(
                            resg[:], res[:],
                            mybir.ActivationFunctionType.Sigmoid)
                        nc.sync.dma_start(y_d.ap()[mb * P:(mb + 1) * P, :],
                                          resg[:])

    nc.compile()
    return nc


_NC_CACHE = None


def _get_program():
    global _NC_CACHE
    if _NC_CACHE is None:
        _NC_CACHE = build_program()
    return _NC_CACHE


def make_in_maps(x, weight, att_vec, adj):
    import ml_dtypes
    x = np.asarray(x, dtype=np.float32)
    weight = np.ascontiguousarray(np.asarray(weight, dtype=np.float32))
    att_vec = np.asarray(att_vec, dtype=np.float32)
    adj8 = np.asarray(adj).astype(np.int8)

    xt = np.ascontiguousarray(x.T.astype(ml_dtypes.float8_e4m3))  # [FIN, N]
    wt = np.ascontiguousarray(weight.T)                           # [FOUT, FIN]
    w8 = weight.astype(ml_dtypes.float8_e4m3)
    a1 = att_vec[:FOUT, 0]
    a2 = att_vec[FOUT:, 0]
    att3 = np.ascontiguousarray(
        np.stack([a1 + a2, a1, a2], axis=1).astype(np.float32))   # [FOUT, 3]
    ebasis = np.array([[0.0, 0.0], [1.0, 0.0], [0.0, 1.0]], np.float32)
    in_maps = []
    for c in range(NCORES):
        g = 1.0 if c < 4 else 0.0
        gcol = np.empty((P, 2), np.float32)
        gcol[:, 0] = g
        gcol[:, 1] = 1.0 - g
        selg = np.zeros((P, MB, HKC), np.float32)
        for bp in range(MB):
            selg[:, bp, (c * MB + bp) % HKC] = g
        adjt = np.ascontiguousarray(adj8[c * RPC:(c + 1) * RPC, :].T)
        in_maps.append({
            "xt": xt,
            "wt": wt,
            "w8": w8,
            "att3": att3,
            "adjt": adjt,
            "gcol": gcol,
            "selg": selg,
            "ebasis": ebasis,
        })
    return in_maps


def kernel(x, weight, att_vec, adj, _trace=False, _trace_kwargs=None):
    nc = _get_program()
    in_maps = make_in_maps(x, weight, att_vec, adj)
    r = run_bass_kernel_spmd(nc, in_maps, core_ids=list(range(NCORES)),
                             trace=_trace, **(_trace_kwargs or {}))
    y = np.concatenate([r.results[c]["y"] for c in range(NCORES)], axis=0)
    kernel.last_results = r
    return y.astype(np.float32)



# revision 7
# speedup vs baseline: 1.2355x; 1.2355x over previous
"""GAT layer (nn_GAT_layer_67619965108552) as a Trainium2 Bass/Tile SPMD kernel.

Same collapsed math as the previous version (see prep_check.py for the numpy
emulation), restructured for speed:

  * The separate U-pass is gone: the score vectors ride as 6 extra rhs columns
    ([wam_hi | wam_lo] fp8 hi/lo split for precision) on the out-pass matmuls,
    so one pass over xT produces out AND [u, s1, s2] per node.
  * Even/odd pair extraction (tt = s1[even]+s2[odd], u pairs for the alphas)
    happens on the PE with a single resident 0/1 stationary E_eo: one N=24
    matmul per 8 blocks into one PSUM bank, whose columns line up so every
    downstream consumer is a uniform stride-6 2D slice. No DRAM roundtrip.
  * All DRAM operands are host-pre-tiled so every DMA descriptor is a 4-8 KB
    contiguous per-partition run (the old layouts were descriptor-rate bound
    at ~1 KB/descriptor).
  * Stage B runs in mb-slab order: adj arrives as 8 slabs of [all j, 128 i],
    each slab's 64 matmuls accumulate Y1/Y2 for one output row-block, and the
    per-block epilogue (alpha combine, normalize, sigmoid, store) overlaps the
    next slab's matmuls. No epilogue tail.
  * Stage B is all-fp8 (adj 0/1 exact; fo = f*out pre-scaled by 1/4 so the
    normalizer ratio is unchanged and values stay far below the TRN fp8e4
    240 cap). Numpy-emulated end-to-end rel err: 2.5e-3.

Sharding: rows of adj (and the output) across 8 cores, 1024 rows each;
x/weight replicated; every core computes the full out = x@W.
"""
import numpy as np

import concourse.bass as bass
import concourse.tile as tile
from concourse import bacc, mybir
from concourse.bass_utils import run_bass_kernel_spmd

F32 = mybir.dt.float32
F32R = mybir.dt.float32r
BF16 = mybir.dt.bfloat16
F8 = mybir.dt.float8e4

N = 8192
FIN = 512
FOUT = 256
P = 128
NB = N // P        # 64 node blocks
NCORES = 8
RPC = N // NCORES  # 1024 output rows per core
MB = RPC // P      # 8 output row blocks per core
NS = 8             # xT slices
SC = 0.25          # fo pre-scale (cancels in the normalizer ratio)


def build_program():
    nc = bacc.Bacc("TRN2", target_bir_lowering=False, debug=False,
                   num_devices=NCORES)

    xt_d = nc.dram_tensor("xt", [NS, P, 4 * 1024], F8, kind="ExternalInput")
    wrhs_d = nc.dram_tensor("wrhs", [P, 4 * 262], F8, kind="ExternalInput")
    geo_d = nc.dram_tensor("geo", [P, 4 * P], F32R, kind="ExternalInput")
    adjt_d = nc.dram_tensor("adjt", [MB, P, NB * P], F8, kind="ExternalInput")
    # gcol cols: [g*SC, (1-g)*SC, 0, 1-g]
    g_d = nc.dram_tensor("gcol", [P, 4], F32, kind="ExternalInput")
    # selg[p, bp, B] = g * (B == 8c + bp)
    selg_d = nc.dram_tensor("selg", [P, MB, 32], F32, kind="ExternalInput")
    y_d = nc.dram_tensor("y", [RPC, FOUT], F32, kind="ExternalOutput")

    with tile.TileContext(nc) as tc:
        with tc.tile_pool(name="const", bufs=1) as constp, \
             tc.tile_pool(name="adjp", bufs=3) as adjp, \
             tc.tile_pool(name="sa", bufs=3) as sa:

            # ---- constants (scalar HWDGE ring; sync ring streams xT) ----
            wrhs = constp.tile([P, 4, 262], F8)
            nc.scalar.dma_start(wrhs[:], wrhs_d.ap().rearrange(
                "p (c v) -> p c v", v=262))
            geo = constp.tile([P, 4, P], F32R)
            nc.scalar.dma_start(geo[:], geo_d.ap().rearrange(
                "p (g q) -> p g q", q=P))
            gcol = constp.tile([P, 4], F32)
            nc.scalar.dma_start(gcol[:], g_d.ap())
            selg = constp.tile([P, MB, 32], F32)
            nc.scalar.dma_start(selg[:], selg_d.ap())

            dumm = constp.tile([1, 2], F32)
            nc.gpsimd.memset(dumm[:], 0.0)

            # persistent staging: out blocks (col 256 preset to 1.0 so the
            # f-scale pass emits the normalizer column for free)
            outb = [constp.tile([P, FOUT + 1], BF16, name=f"outb{b}")
                    for b in range(NB)]
            for b in range(NB):
                nc.gpsimd.memset(outb[b][:, FOUT:FOUT + 1], 1.0)
            fo8 = [constp.tile([P, FOUT + 1], F8, name=f"fo8_{b}")
                   for b in range(NB)]
            uc6 = constp.tile([P, NB, 6], F32)
            ucols = constp.tile([P, NB, 3], F32R)
            fpm = constp.tile([P, 32], F32)
            al1 = constp.tile([P, MB], F32)
            al2 = constp.tile([P, MB], F32)

            # ---- fused out+score pass over the xT stream ----
            with tc.tile_pool(name="xtp", bufs=NS) as xtp, \
                 tc.tile_pool(name="ps_o", bufs=3, space="PSUM") as ps_o, \
                 tc.tile_pool(name="ps_e", bufs=1, space="PSUM") as ps_e:

                # eoX cols: [tts | ae | be], partition-aligned
                eoX = ps_e.tile([P, 96], F32, name="eoX")

                for s in range(NS):
                    xs = xtp.tile([P, 4, 1024], F8, tag="xts", name=f"xt{s}")
                    nc.sync.dma_start(xs[:], xt_d.ap()[s].rearrange(
                        "p (c n) -> p c n", n=1024))
                    for b8 in range(8):
                        b = s * 8 + b8
                        po = ps_o.tile([P, 262], F32, tag="po", name="po")
                        for c in range(4):
                            nc.tensor.matmul(
                                po[:], xs[:, c, b8 * P:(b8 + 1) * P],
                                wrhs[:, c, :], start=(c == 0), stop=(c == 3))
                        if b == 1:
                            # exp-table warm, anchored mid-stream
                            nc.scalar.activation(
                                dumm[:, 0:1], ucols[0:1, 0, 0:1],
                                mybir.ActivationFunctionType.Exp)
                        if b % 2 == 0:
                            nc.scalar.copy(outb[b][:, 0:FOUT], po[:, 0:FOUT])
                            nc.vector.tensor_copy(uc6[:, b, :],
                                                  po[:, 256:262])
                        else:
                            nc.vector.tensor_copy(outb[b][:, 0:FOUT],
                                                  po[:, 0:FOUT])
                            nc.scalar.copy(uc6[:, b, :], po[:, 256:262])
                    sl = slice(s * 8, (s + 1) * 8)
                    nc.vector.tensor_add(ucols[:, sl, :], uc6[:, sl, 0:3],
                                         uc6[:, sl, 3:6])

                # ---- extraction: selector matmuls, PSUM-accumulated ----
                # G1 even->top, G2 odd->top, G3 even->bot, G4 odd->bot
                # tts = G1.s1e + G2.s2e + G3.s1o + G4.s2o
                nc.tensor.matmul(eoX[:, 0:32], geo[:, 0, :],
                                 ucols[:, 0::2, 1], start=True, stop=False)
                nc.tensor.matmul(eoX[:, 0:32], geo[:, 1, :],
                                 ucols[:, 0::2, 2], start=False, stop=False)
                nc.tensor.matmul(eoX[:, 0:32], geo[:, 2, :],
                                 ucols[:, 1::2, 1], start=False, stop=False)
                nc.tensor.matmul(eoX[:, 0:32], geo[:, 3, :],
                                 ucols[:, 1::2, 2], start=False, stop=True)
                # ae = G1.ue + G3.uo ; be = G2.ue + G4.uo
                nc.tensor.matmul(eoX[:, 32:64], geo[:, 0, :],
                                 ucols[:, 0::2, 0], start=True, stop=False)
                nc.tensor.matmul(eoX[:, 32:64], geo[:, 2, :],
                                 ucols[:, 1::2, 0], start=False, stop=True)
                nc.tensor.matmul(eoX[:, 64:96], geo[:, 1, :],
                                 ucols[:, 0::2, 0], start=True, stop=False)
                nc.tensor.matmul(eoX[:, 64:96], geo[:, 3, :],
                                 ucols[:, 1::2, 0], start=False, stop=True)

                # fpm = SC * (g + (1-g)*exp(lrelu(tts)))
                lrt = sa.tile([P, 32], F32, tag="lrt", name="lrt")
                nc.vector.tensor_scalar_mul(lrt[:], eoX[:, 0:32], 0.01)
                nc.vector.tensor_max(lrt[:], eoX[:, 0:32], lrt[:])
                ext = sa.tile([P, 32], F32, tag="ext", name="ext")
                nc.scalar.activation(ext[:], lrt[:],
                                     mybir.ActivationFunctionType.Exp)
                nc.vector.tensor_scalar(fpm[:], ext[:], gcol[:, 1:2],
                                        gcol[:, 0:1],
                                        op0=mybir.AluOpType.mult,
                                        op1=mybir.AluOpType.add)
                # sigmoid-table warm, anchored on fpm
                nc.scalar.activation(dumm[:, 1:2], fpm[0:1, 0:1],
                                     mybir.ActivationFunctionType.Sigmoid)

                # alphas: aev/bev = exp(lrelu(ae/be)), al = selg-reduce + 1-g
                lra = sa.tile([P, 64], F32, tag="lra", name="lra")
                aeb = sa.tile([P, 64], F32, tag="aeb", name="aeb")
                nc.vector.tensor_copy(aeb[:, 0:32], eoX[:, 32:64])
                nc.vector.tensor_copy(aeb[:, 32:64], eoX[:, 64:96])
                nc.vector.tensor_scalar_mul(lra[:], aeb[:], 0.01)
                nc.vector.tensor_max(lra[:], aeb[:], lra[:])
                nc.scalar.activation(aeb[:], lra[:],
                                     mybir.ActivationFunctionType.Exp)
                for bp in range(MB):
                    m1 = sa.tile([P, 32], F32, tag="alm", name="alm1")
                    nc.vector.tensor_mul(m1[:], aeb[:, 0:32], selg[:, bp, :])
                    nc.vector.tensor_reduce(al1[:, bp:bp + 1], m1[:],
                                            axis=mybir.AxisListType.X,
                                            op=mybir.AluOpType.add)
                    m2 = sa.tile([P, 32], F32, tag="alm", name="alm2")
                    nc.vector.tensor_mul(m2[:], aeb[:, 32:64], selg[:, bp, :])
                    nc.vector.tensor_reduce(al2[:, bp:bp + 1], m2[:],
                                            axis=mybir.AxisListType.X,
                                            op=mybir.AluOpType.add)
                nc.vector.tensor_scalar_add(al1[:], al1[:], gcol[:, 3:4])
                nc.vector.tensor_scalar_add(al2[:], al2[:], gcol[:, 3:4])

            # ---- f-scale pass (jc order = stage-B consumption order) ----
            with tc.tile_pool(name="ps_y", bufs=4, space="PSUM") as ps_y, \
                 tc.tile_pool(name="comb", bufs=3) as comb:

                for jc in range(NB):
                    kc = jc % 32
                    if jc % 2 == 0:
                        nc.scalar.activation(fo8[jc][:], outb[jc][:],
                                             mybir.ActivationFunctionType.Copy,
                                             scale=fpm[:, kc:kc + 1])
                    else:
                        nc.vector.tensor_scalar_mul(fo8[jc][:], outb[jc][:],
                                                    fpm[:, kc:kc + 1])

                # ---- adj slabs (SWDGE; early ones gated on out-pass
                # progress so they don't steal HBM from the xT stream) ----
                def slab_load(mb, marker=None):
                    t = adjp.tile([P, NB, P], F8, tag="slab", name=f"slab{mb}")
                    if marker is not None:
                        nc.scalar.copy(t[0:1, 0, 0:1], marker)
                    nc.gpsimd.dma_start(t[:], adjt_d.ap()[mb].rearrange(
                        "p (j i) -> p j i", i=P))
                    return t

                slabs = [slab_load(0, ucols[0:1, 8, 0:1]),
                         slab_load(1, ucols[0:1, 24, 0:1]),
                         slab_load(2, ucols[0:1, 40, 0:1])]
                slabs += [slab_load(mb) for mb in range(3, MB)]

                # ---- stage B: per row-block accumulate + inline epilogue ----
                for mb in range(MB):
                    at = slabs[mb]
                    ya = ps_y.tile([P, FOUT + 1], F32, tag="ya", name="ya")
                    yb = ps_y.tile([P, FOUT + 1], F32, tag="yb", name="yb")
                    for jc in range(NB):
                        dst = ya if jc < 32 else yb
                        nc.tensor.matmul(dst[:], at[:, jc, :], fo8[jc][:],
                                         start=(jc % 32 == 0),
                                         stop=(jc % 32 == 31))
                    za = comb.tile([P, FOUT + 1], F32, tag="za", name="za")
                    nc.scalar.activation(za[:], ya[:],
                                         mybir.ActivationFunctionType.Copy,
                                         scale=al1[:, mb:mb + 1])
                    t2 = comb.tile([P, FOUT + 1], F32, tag="t2", name="t2")
                    nc.vector.tensor_scalar_mul(t2[:], yb[:],
                                                al2[:, mb:mb + 1])
                    z2 = comb.tile([P, FOUT + 1], F32, tag="z2", name="z2")
                    nc.vector.tensor_add(z2[:], za[:], t2[:])
                    rec = comb.tile([P, 1], F32, tag="rec", name="rec")
                    nc.vector.reciprocal(rec[:], z2[:, FOUT:FOUT + 1])
                    res = comb.tile([P, FOUT], F32, tag="res", name="res")
                    nc.vector.tensor_scalar_mul(res[:], z2[:, :FOUT], rec[:])
                    resg = comb.tile([P, FOUT], F32, tag="resg", name="resg")
                    nc.scalar.activation(resg[:], res[:],
                                         mybir.ActivationFunctionType.Sigmoid)
                    nc.sync.dma_start(y_d.ap()[mb * P:(mb + 1) * P, :],
                                      resg[:])

    nc.compile()
    return nc


_NC_CACHE = None


def _get_program():
    global _NC_CACHE
    if _NC_CACHE is None:
        _NC_CACHE = build_program()
    return _NC_CACHE


def make_in_maps(x, weight, att_vec, adj):
    import ml_dtypes
    f8 = ml_dtypes.float8_e4m3
    x = np.asarray(x, dtype=np.float32)
    weight = np.ascontiguousarray(np.asarray(weight, dtype=np.float32))
    att_vec = np.asarray(att_vec, dtype=np.float32)

    # xt[s, p, c*1024+n] = x[s*1024+n, c*128+p]
    x8 = x.astype(f8)
    xt = np.ascontiguousarray(
        x8.reshape(NS, 1024, 4, P).transpose(0, 3, 2, 1)).reshape(NS, P, 4096)

    a1 = att_vec[:FOUT, 0]
    a2 = att_vec[FOUT:, 0]
    att3 = np.stack([a1 + a2, a1, a2], axis=1).astype(np.float32)
    wam = (weight @ att3).astype(np.float32)            # [FIN, 3]
    wam_hi = wam.astype(f8)
    wam_lo = (wam - wam_hi.astype(np.float32)).astype(f8)
    wcat = np.concatenate([weight.astype(f8),
                           wam_hi, wam_lo], axis=1)     # [FIN, 262]
    wrhs = np.ascontiguousarray(
        wcat.reshape(4, P, 262).transpose(1, 0, 2)).reshape(P, 4 * 262)

    geo = np.zeros((4, P, P), np.float32)   # [g, p, q]
    q = np.arange(64)
    geo[0, 2 * q, q] = 1.0          # even -> top
    geo[1, 2 * q + 1, q] = 1.0      # odd  -> top
    geo[2, 2 * q, q + 64] = 1.0     # even -> bot
    geo[3, 2 * q + 1, q + 64] = 1.0  # odd -> bot
    geo = np.ascontiguousarray(geo.transpose(1, 0, 2)).reshape(P, 4 * P)

    adj8 = np.asarray(adj, dtype=np.int8).astype(f8)
    in_maps = []
    for c in range(NCORES):
        g = 1.0 if c < 4 else 0.0
        gcol = np.empty((P, 4), np.float32)
        gcol[:, 0] = g * SC
        gcol[:, 1] = (1.0 - g) * SC
        gcol[:, 2] = 0.0
        gcol[:, 3] = 1.0 - g
        selg = np.zeros((P, MB, 32), np.float32)
        for bp in range(MB):
            selg[:, bp, (c * MB + bp) % 32] = g
        # adjt[mb, p, jc*128+i] = adj[c*1024 + mb*128 + i, jc*128 + p]
        A = adj8[c * RPC:(c + 1) * RPC, :]
        adjt = np.ascontiguousarray(
            A.reshape(MB, P, NB, P).transpose(0, 3, 2, 1)).reshape(
                MB, P, NB * P)
        in_maps.append({
            "xt": xt,
            "wrhs": wrhs,
            "geo": geo,
            "adjt": adjt,
            "gcol": gcol,
            "selg": selg,
        })
    return in_maps


def kernel(x, weight, att_vec, adj, _trace=False, _trace_kwargs=None):
    nc = _get_program()
    in_maps = make_in_maps(x, weight, att_vec, adj)
    r = run_bass_kernel_spmd(nc, in_maps, core_ids=list(range(NCORES)),
                             trace=_trace, **(_trace_kwargs or {}))
    y = np.concatenate([r.results[c]["y"] for c in range(NCORES)], axis=0)
    kernel.last_results = r
    return y.astype(np.float32)


# revision 9
# speedup vs baseline: 1.2636x; 1.0227x over previous
"""GAT layer (nn_GAT_layer_67619965108552) as a Trainium2 Bass/Tile SPMD kernel.

Same collapsed math as the previous version (see prep_check.py for the numpy
emulation), restructured for speed:

  * The separate U-pass is gone: the score vectors ride as 6 extra rhs columns
    ([wam_hi | wam_lo] fp8 hi/lo split for precision) on the out-pass matmuls,
    so one pass over xT produces out AND [u, s1, s2] per node.
  * Even/odd pair extraction (tt = s1[even]+s2[odd], u pairs for the alphas)
    happens on the PE with a single resident 0/1 stationary E_eo: one N=24
    matmul per 8 blocks into one PSUM bank, whose columns line up so every
    downstream consumer is a uniform stride-6 2D slice. No DRAM roundtrip.
  * All DRAM operands are host-pre-tiled so every DMA descriptor is a 4-8 KB
    contiguous per-partition run (the old layouts were descriptor-rate bound
    at ~1 KB/descriptor).
  * Stage B runs in mb-slab order: adj arrives as 8 slabs of [all j, 128 i],
    each slab's 64 matmuls accumulate Y1/Y2 for one output row-block, and the
    per-block epilogue (alpha combine, normalize, sigmoid, store) overlaps the
    next slab's matmuls. No epilogue tail.
  * Stage B is all-fp8 (adj 0/1 exact; fo = f*out pre-scaled by 1/4 so the
    normalizer ratio is unchanged and values stay far below the TRN fp8e4
    240 cap). Numpy-emulated end-to-end rel err: 2.5e-3.

Sharding: rows of adj (and the output) across 8 cores, 1024 rows each;
x/weight replicated; every core computes the full out = x@W.
"""
import numpy as np

import concourse.bass as bass
import concourse.tile as tile
from concourse import bacc, mybir
from concourse.bass_utils import run_bass_kernel_spmd

F32 = mybir.dt.float32
F32R = mybir.dt.float32r
BF16 = mybir.dt.bfloat16
F8 = mybir.dt.float8e4

N = 8192
FIN = 512
FOUT = 256
P = 128
NB = N // P        # 64 node blocks
NCORES = 8
RPC = N // NCORES  # 1024 output rows per core
MB = RPC // P      # 8 output row blocks per core
NS = 8             # xT slices
SC = 0.25          # fo pre-scale (cancels in the normalizer ratio)


def build_program():
    nc = bacc.Bacc("TRN2", target_bir_lowering=False, debug=False,
                   num_devices=NCORES)

    xt_d = nc.dram_tensor("xt", [NS, P, 4 * 1024], F8, kind="ExternalInput")
    wrhs_d = nc.dram_tensor("wrhs", [P, 4 * 262], F8, kind="ExternalInput")
    geo_d = nc.dram_tensor("geo", [P, 4 * P], F32R, kind="ExternalInput")
    adjt_d = nc.dram_tensor("adjt", [MB, P, NB * P], F8, kind="ExternalInput")
    # gcol cols: [g*SC, (1-g)*SC, 0, 1-g]
    g_d = nc.dram_tensor("gcol", [P, 4], F32, kind="ExternalInput")
    # selg[p, bp, B] = g * (B == 8c + bp)
    selg_d = nc.dram_tensor("selg", [P, MB, 32], F32, kind="ExternalInput")
    y_d = nc.dram_tensor("y", [RPC, FOUT], F32, kind="ExternalOutput")

    with tile.TileContext(nc) as tc:
        with tc.tile_pool(name="const", bufs=1) as constp, \
             tc.tile_pool(name="adjp", bufs=3) as adjp, \
             tc.tile_pool(name="sa", bufs=3) as sa:

            # ---- constants (scalar HWDGE ring; sync ring streams xT) ----
            wrhs = constp.tile([P, 4, 262], F8)
            nc.scalar.dma_start(wrhs[:], wrhs_d.ap().rearrange(
                "p (c v) -> p c v", v=262))
            geo = constp.tile([P, 4, P], F32R)
            nc.scalar.dma_start(geo[:], geo_d.ap().rearrange(
                "p (g q) -> p g q", q=P))
            gcol = constp.tile([P, 4], F32)
            nc.scalar.dma_start(gcol[:], g_d.ap())
            selg = constp.tile([P, MB, 32], F32)
            nc.scalar.dma_start(selg[:], selg_d.ap())

            dumm = constp.tile([1, 2], F32)
            nc.gpsimd.memset(dumm[:], 0.0)

            # persistent staging: out blocks (col 256 preset to 1.0 so the
            # f-scale pass emits the normalizer column for free)
            outb = [constp.tile([P, FOUT + 1], BF16, name=f"outb{b}")
                    for b in range(NB)]
            for b in range(NB):
                nc.gpsimd.memset(outb[b][:, FOUT:FOUT + 1], 1.0)
            fo8 = [constp.tile([P, FOUT + 1], F8, name=f"fo8_{b}")
                   for b in range(NB)]
            uc6 = constp.tile([P, NB, 6], F32)
            ucols = constp.tile([P, NB, 3], F32R)
            fpm = constp.tile([P, 32], F32)
            al1 = constp.tile([P, MB], F32)
            al2 = constp.tile([P, MB], F32)

            # ---- fused out+score pass over the xT stream ----
            with tc.tile_pool(name="xtp", bufs=NS) as xtp, \
                 tc.tile_pool(name="ps_o", bufs=3, space="PSUM") as ps_o, \
                 tc.tile_pool(name="ps_e", bufs=1, space="PSUM") as ps_e:

                # eoX cols: [tts 32 | ae 32 | be 32], partition-aligned
                eoX = ps_e.tile([P, 96], F32, name="eoX")

                def extract_half(h):
                    # rhs: even/odd blocks of half h; dst cols 16h:16h+16
                    ev = ucols[:, 32 * h:32 * h + 32:2, :]
                    od = ucols[:, 32 * h + 1:32 * h + 32:2, :]
                    for base, pairs in ((0, ((0, ev, 1), (1, ev, 2),
                                             (2, od, 1), (3, od, 2))),
                                        (32, ((0, ev, 0), (2, od, 0))),
                                        (64, ((1, ev, 0), (3, od, 0)))):
                        dst = eoX[:, base + 16 * h:base + 16 * h + 16]
                        for k, (g, rr, v) in enumerate(pairs):
                            nc.tensor.matmul(dst, geo[:, g, :], rr[:, :, v],
                                             start=(k == 0),
                                             stop=(k == len(pairs) - 1))

                def fpm_half(h):
                    hs = slice(16 * h, 16 * h + 16)
                    lrt = sa.tile([P, 16], F32, tag="lrt", name="lrt")
                    nc.vector.tensor_scalar_mul(lrt[:], eoX[:, hs], 0.01)
                    nc.vector.tensor_max(lrt[:], eoX[:, hs], lrt[:])
                    ext = sa.tile([P, 16], F32, tag="ext", name="ext")
                    nc.scalar.activation(ext[:], lrt[:],
                                         mybir.ActivationFunctionType.Exp)
                    nc.vector.tensor_scalar(fpm[:, hs], ext[:], gcol[:, 1:2],
                                            gcol[:, 0:1],
                                            op0=mybir.AluOpType.mult,
                                            op1=mybir.AluOpType.add)

                def scale_one(jc):
                    kc = jc % 32
                    if jc % 2 == 0:
                        nc.scalar.activation(
                            fo8[jc][:], outb[jc][:],
                            mybir.ActivationFunctionType.Copy,
                            scale=fpm[:, kc:kc + 1])
                    else:
                        nc.vector.tensor_scalar_mul(fo8[jc][:], outb[jc][:],
                                                    fpm[:, kc:kc + 1])

                def half_jcs(h):
                    return list(range(16 * h, 16 * h + 16)) + \
                        list(range(32 + 16 * h, 32 + 16 * h + 16))

                pend_scale = []
                for s in range(NS):
                    if s == 5:
                        pend_scale = half_jcs(0)
                    xs = xtp.tile([P, 4096], F8, tag="xts", name=f"xt{s}")
                    nc.sync.dma_start(xs[:], xt_d.ap()[s])
                    for b8 in range(8):
                        b = s * 8 + b8
                        po = ps_o.tile([P, 262], F32, tag="po", name="po")
                        for c in range(4):
                            nc.tensor.matmul(
                                po[:],
                                xs[:, c * 1024 + b8 * P:
                                   c * 1024 + (b8 + 1) * P],
                                wrhs[:, c, :], start=(c == 0), stop=(c == 3))
                        if b == 1:
                            # exp-table warm, anchored mid-stream
                            nc.scalar.activation(
                                dumm[:, 0:1], ucols[0:1, 0, 0:1],
                                mybir.ActivationFunctionType.Exp)
                        if b == 34:
                            # first-half extraction rides mid-stream (deps
                            # on slices 0-3 are settled by now)
                            extract_half(0)
                            fpm_half(0)
                        if b >= 36 and pend_scale:
                            # drip fo8 scales through the copy stream so the
                            # FIFO ACT/DVE queues never stall the PE
                            scale_one(pend_scale.pop(0))
                        if b % 2 == 0:
                            nc.scalar.copy(outb[b][:, 0:FOUT], po[:, 0:FOUT])
                            nc.vector.tensor_copy(uc6[:, b, :],
                                                  po[:, 256:262])
                        else:
                            nc.vector.tensor_copy(outb[b][:, 0:FOUT],
                                                  po[:, 0:FOUT])
                            nc.scalar.copy(uc6[:, b, :], po[:, 256:262])
                    sl = slice(s * 8, (s + 1) * 8)
                    nc.vector.tensor_add(ucols[:, sl, :], uc6[:, sl, 0:3],
                                         uc6[:, sl, 3:6])
                for jc in pend_scale:
                    scale_one(jc)
                extract_half(1)
                fpm_half(1)
                for jc in half_jcs(1):
                    scale_one(jc)
                # sigmoid-table warm, anchored on fpm
                nc.scalar.activation(dumm[:, 1:2], fpm[0:1, 0:1],
                                     mybir.ActivationFunctionType.Sigmoid)

                # alphas: aev/bev = exp(lrelu(ae/be)), al = selg-reduce + 1-g
                lra = sa.tile([P, 64], F32, tag="lra", name="lra")
                aeb = sa.tile([P, 64], F32, tag="aeb", name="aeb")
                nc.vector.tensor_copy(aeb[:, 0:32], eoX[:, 32:64])
                nc.vector.tensor_copy(aeb[:, 32:64], eoX[:, 64:96])
                nc.vector.tensor_scalar_mul(lra[:], aeb[:], 0.01)
                nc.vector.tensor_max(lra[:], aeb[:], lra[:])
                nc.scalar.activation(aeb[:], lra[:],
                                     mybir.ActivationFunctionType.Exp)
                for bp in range(MB):
                    m1 = sa.tile([P, 32], F32, tag="alm", name="alm1")
                    nc.vector.tensor_mul(m1[:], aeb[:, 0:32], selg[:, bp, :])
                    nc.vector.tensor_reduce(al1[:, bp:bp + 1], m1[:],
                                            axis=mybir.AxisListType.X,
                                            op=mybir.AluOpType.add)
                    m2 = sa.tile([P, 32], F32, tag="alm", name="alm2")
                    nc.vector.tensor_mul(m2[:], aeb[:, 32:64], selg[:, bp, :])
                    nc.vector.tensor_reduce(al2[:, bp:bp + 1], m2[:],
                                            axis=mybir.AxisListType.X,
                                            op=mybir.AluOpType.add)
                nc.vector.tensor_scalar_add(al1[:], al1[:], gcol[:, 3:4])
                nc.vector.tensor_scalar_add(al2[:], al2[:], gcol[:, 3:4])

            # ---- f-scale pass (jc order = stage-B consumption order) ----
            with tc.tile_pool(name="ps_y", bufs=4, space="PSUM") as ps_y, \
                 tc.tile_pool(name="comb", bufs=3) as comb:

                # ---- adj slabs (SWDGE; early ones gated on out-pass
                # progress so they don't steal HBM from the xT stream) ----
                def slab_load(mb, marker=None):
                    t = adjp.tile([P, NB * P], F8, tag="slab",
                                  name=f"slab{mb}")
                    if marker is not None:
                        nc.scalar.copy(t[0:1, 0:1], marker)
                    nc.gpsimd.dma_start(t[:], adjt_d.ap()[mb])
                    return t

                slabs = [slab_load(0, ucols[0:1, 8, 0:1]),
                         slab_load(1, ucols[0:1, 24, 0:1]),
                         slab_load(2, ucols[0:1, 40, 0:1])]
                slabs += [slab_load(mb) for mb in range(3, MB)]

                # ---- stage B: per row-block accumulate + inline epilogue ----
                for mb in range(MB):
                    at = slabs[mb]
                    ya = ps_y.tile([P, FOUT + 1], F32, tag="ya", name="ya")
                    yb = ps_y.tile([P, FOUT + 1], F32, tag="yb", name="yb")
                    for jc in range(NB):
                        dst = ya if jc < 32 else yb
                        nc.tensor.matmul(dst[:], at[:, jc * P:(jc + 1) * P], fo8[jc][:],
                                         start=(jc % 32 == 0),
                                         stop=(jc % 32 == 31))
                    za = comb.tile([P, FOUT + 1], F32, tag="za", name="za")
                    nc.scalar.activation(za[:], ya[:],
                                         mybir.ActivationFunctionType.Copy,
                                         scale=al1[:, mb:mb + 1])
                    t2 = comb.tile([P, FOUT + 1], F32, tag="t2", name="t2")
                    nc.vector.tensor_scalar_mul(t2[:], yb[:],
                                                al2[:, mb:mb + 1])
                    z2 = comb.tile([P, FOUT + 1], F32, tag="z2", name="z2")
                    nc.vector.tensor_add(z2[:], za[:], t2[:])
                    rec = comb.tile([P, 1], F32, tag="rec", name="rec")
                    nc.vector.reciprocal(rec[:], z2[:, FOUT:FOUT + 1])
                    res = comb.tile([P, FOUT], F32, tag="res", name="res")
                    nc.vector.tensor_scalar_mul(res[:], z2[:, :FOUT], rec[:])
                    resg = comb.tile([P, FOUT], F32, tag="resg", name="resg")
                    nc.scalar.activation(resg[:], res[:],
                                         mybir.ActivationFunctionType.Sigmoid)
                    nc.sync.dma_start(y_d.ap()[mb * P:(mb + 1) * P, :],
                                      resg[:])

    nc.compile()
    return nc


_NC_CACHE = None


def _get_program():
    global _NC_CACHE
    if _NC_CACHE is None:
        _NC_CACHE = build_program()
    return _NC_CACHE


def make_in_maps(x, weight, att_vec, adj):
    import ml_dtypes
    f8 = ml_dtypes.float8_e4m3
    x = np.asarray(x, dtype=np.float32)
    weight = np.ascontiguousarray(np.asarray(weight, dtype=np.float32))
    att_vec = np.asarray(att_vec, dtype=np.float32)

    # xt[s, p, c*1024+n] = x[s*1024+n, c*128+p]
    x8 = x.astype(f8)
    xt = np.ascontiguousarray(
        x8.reshape(NS, 1024, 4, P).transpose(0, 3, 2, 1)).reshape(NS, P, 4096)

    a1 = att_vec[:FOUT, 0]
    a2 = att_vec[FOUT:, 0]
    att3 = np.stack([a1 + a2, a1, a2], axis=1).astype(np.float32)
    wam = (weight @ att3).astype(np.float32)            # [FIN, 3]
    wam_hi = wam.astype(f8)
    wam_lo = (wam - wam_hi.astype(np.float32)).astype(f8)
    wcat = np.concatenate([weight.astype(f8),
                           wam_hi, wam_lo], axis=1)     # [FIN, 262]
    wrhs = np.ascontiguousarray(
        wcat.reshape(4, P, 262).transpose(1, 0, 2)).reshape(P, 4 * 262)

    geo = np.zeros((4, P, P), np.float32)   # [g, p, q]
    q = np.arange(64)
    geo[0, 2 * q, q] = 1.0          # even -> top
    geo[1, 2 * q + 1, q] = 1.0      # odd  -> top
    geo[2, 2 * q, q + 64] = 1.0     # even -> bot
    geo[3, 2 * q + 1, q + 64] = 1.0  # odd -> bot
    geo = np.ascontiguousarray(geo.transpose(1, 0, 2)).reshape(P, 4 * P)

    adj8 = np.asarray(adj, dtype=np.int8).astype(f8)
    in_maps = []
    for c in range(NCORES):
        g = 1.0 if c < 4 else 0.0
        gcol = np.empty((P, 4), np.float32)
        gcol[:, 0] = g * SC
        gcol[:, 1] = (1.0 - g) * SC
        gcol[:, 2] = 0.0
        gcol[:, 3] = 1.0 - g
        selg = np.zeros((P, MB, 32), np.float32)
        for bp in range(MB):
            selg[:, bp, (c * MB + bp) % 32] = g
        # adjt[mb, p, jc*128+i] = adj[c*1024 + mb*128 + i, jc*128 + p]
        A = adj8[c * RPC:(c + 1) * RPC, :]
        adjt = np.ascontiguousarray(
            A.reshape(MB, P, NB, P).transpose(0, 3, 2, 1)).reshape(
                MB, P, NB * P)
        in_maps.append({
            "xt": xt,
            "wrhs": wrhs,
            "geo": geo,
            "adjt": adjt,
            "gcol": gcol,
            "selg": selg,
        })
    return in_maps


def kernel(x, weight, att_vec, adj, _trace=False, _trace_kwargs=None):
    nc = _get_program()
    in_maps = make_in_maps(x, weight, att_vec, adj)
    r = run_bass_kernel_spmd(nc, in_maps, core_ids=list(range(NCORES)),
                             trace=_trace, **(_trace_kwargs or {}))
    y = np.concatenate([r.results[c]["y"] for c in range(NCORES)], axis=0)
    kernel.last_results = r
    return y.astype(np.float32)
